# revision 1
# baseline (speedup 1.0000x reference)
"""Trainium2 Bass kernel for a 3-layer LSTM recurrent encoder.

Contract: kernel(**inputs) takes FULL inputs (as produced by
setup_inputs()) and returns the FULL output [256, 128, 16] fp32.

Strategy: data-parallel over the batch (256 tracks -> 8 cores x 32).
Per core, cuDNN-style layer phases:
  P0:   xpT = relu(proj_w.T @ xT + proj_b)     (stored transposed [512, 4096])
  per layer l:
    P1a: in-place relu on the previous layer's hT_all (skipped for l=0)
    P1b: Gx = inp @ Wx[l] + b[l]  batched over all 4096 tokens -> DRAM
    P1c: 128 serial steps; gates = hT_prev.T @ Wh[l] + Gx[t] via PSUM
         accumulation (Gx injected with an identity-stationary matmul),
         gate math on ACT/DVE, h transposed back via PE into hT_all.
  P2:   yT = Wout.T @ h2T_all + out_b
All matmuls run in float32r (TF32-like, 1 cycle/row at N>=256).
Gate columns are host-permuted into per-128-block [i|f|o|g] order so one
sigmoid covers i,f,o contiguously; the +1.0 forget bias is folded into b.
"""

import sys

sys.path.insert(0, "/opt/trn_rl_repo")

import numpy as np

import concourse.bass as bass
import concourse.bacc as bacc
import concourse.mybir as mybir
import concourse.tile as tile
from concourse.bass_utils import run_bass_kernel_spmd
from concourse.masks import make_identity

AF = mybir.ActivationFunctionType
F32 = mybir.dt.float32
F32R = mybir.dt.float32r
BF16 = mybir.dt.bfloat16

B, T, F_IN = 256, 128, 16
H, L, OUT = 512, 3, 16
NCORES = 8
BL = B // NCORES          # 32 tracks per core
R = BL * T                # 4096 tokens per core
H4 = 4 * H                # 2048 gate columns
KT = H // 128             # 4 K-tiles of the hidden dim
NCH = H4 // 512           # 4 gate chunks of 512

_PROG = None  # (nc, in_names) cache so repeated kernel() calls reuse the NEFF


def _gate_perm():
    """Column permutation: per 128-block n -> [i_n, f_n, o_n, g_n]."""
    idx = []
    for n in range(KT):
        blk = np.arange(n * 128, (n + 1) * 128)
        idx.append(0 * H + blk)  # i
        idx.append(2 * H + blk)  # f
        idx.append(3 * H + blk)  # o
        idx.append(1 * H + blk)  # g
    return np.concatenate(idx)


def _build():
    nc = bacc.Bacc("TRN2", target_bir_lowering=False, debug=False,
                   num_devices=NCORES)

    # ---- DRAM I/O (per-core shapes; float32r where fed to the PE) ----
    xT_d = nc.dram_tensor("xT", [F_IN, R], F32R, kind="ExternalInput").ap()
    pw_d = nc.dram_tensor("pw", [F_IN, H], F32R, kind="ExternalInput").ap()
    pb_d = nc.dram_tensor("pb", [H, 1], F32, kind="ExternalInput").ap()
    wx_d = nc.dram_tensor("wx", [L, H, H4], F32R, kind="ExternalInput").ap()
    wh_d = nc.dram_tensor("wh", [L, H, H4], F32R, kind="ExternalInput").ap()
    bi_d = nc.dram_tensor("bi", [L, H4], F32R, kind="ExternalInput").ap()
    wo_d = nc.dram_tensor("wo", [H, OUT], F32R, kind="ExternalInput").ap()
    ob_d = nc.dram_tensor("ob", [OUT, 1], F32, kind="ExternalInput").ap()
    yT_d = nc.dram_tensor("yT", [OUT, R], F32, kind="ExternalOutput").ap()
    gx_d = nc.dram_tensor("gx", [R, H4], F32R, kind="Internal").ap()

    with tile.TileContext(nc) as tc:
        const = tc.alloc_tile_pool(name="const", bufs=1)
        idf = const.tile([32, 32], F32, tag="idf")
        make_identity(nc, idf)
        ident = const.tile([32, 32], F32R, tag="ident")
        nc.vector.tensor_copy(ident[:], idf[:])
        z32f = const.tile([128, 32], F32, tag="z32f")
        nc.vector.memset(z32f, 0.0)
        z32 = const.tile([128, 32], F32R, tag="z32")
        nc.vector.tensor_copy(z32[:], z32f[:])

        hallA = tc.alloc_tile_pool(name="hallA", bufs=1)
        hallB = tc.alloc_tile_pool(name="hallB", bufs=1)
        ctxpools = [const, hallA, hallB]
        A = [hallA.tile([128, R], F32R, tag=f"A{k}", name=f"A{k}") for k in range(KT)]
        Bt = [hallB.tile([128, R], F32R, tag=f"B{k}", name=f"B{k}") for k in range(KT)]

        # ---- P0: projection -> A holds xpT ----
        with tc.tile_pool(name="p0", bufs=1) as p0, \
             tc.tile_pool(name="p0b", bufs=4) as p0b, \
             tc.tile_pool(name="p0ps", bufs=4, space="PSUM") as p0ps:
            xTt = p0.tile([F_IN, R], F32R, tag="xTt")
            nc.sync.dma_start(xTt[:], xT_d)
            pwt = p0.tile([F_IN, H], F32R, tag="pwt")
            nc.sync.dma_start(pwt[:], pw_d)
            for m in range(KT):
                pbt = p0b.tile([128, 1], F32)
                nc.sync.dma_start(pbt[:], pb_d[m * 128:(m + 1) * 128, :])
                for n in range(R // 512):
                    ps = p0ps.tile([128, 512], F32)
                    nc.tensor.matmul(ps[:], pwt[:, m * 128:(m + 1) * 128],
                                     xTt[:, n * 512:(n + 1) * 512],
                                     start=True, stop=True)
                    nc.scalar.activation(A[m][:, n * 512:(n + 1) * 512],
                                         ps[:], AF.Relu, bias=pbt[:])

        # ---- layers ----
        for l in range(L):
            with tc.tile_pool(name=f"bias{l}", bufs=1) as bp:
                bias_bc = bp.tile([128, H4], F32R, tag="bias_bc")
                nc.gpsimd.dma_start(
                    bias_bc[:],
                    bass.AP(tensor=bi_d.tensor, offset=l * H4,
                            ap=[[0, 128], [1, H4]]))

                if l > 0:
                    for k in range(KT):
                        nc.scalar.activation(A[k][:], A[k][:], AF.Relu)

                # P1b: batched Gx = inp @ Wx[l] + b[l] -> gx_d
                with tc.tile_pool(name=f"wx{l}", bufs=1) as wxp, \
                     tc.tile_pool(name=f"gxs{l}", bufs=4) as gxs, \
                     tc.tile_pool(name=f"bps{l}", bufs=4, space="PSUM") as bps:
                    wx = []
                    for k in range(KT):
                        w = wxp.tile([128, H4], F32R, tag=f"wx{k}", name=f"wxt{l}_{k}")
                        nc.sync.dma_start(w[:], wx_d[l, k * 128:(k + 1) * 128, :])
                        wx.append(w)
                    for m in range(R // 128):
                        for n in range(NCH):
                            ps = bps.tile([128, 512], F32)
                            for k in range(KT):
                                nc.tensor.matmul(
                                    ps[:], A[k][:, m * 128:(m + 1) * 128],
                                    wx[k][:, n * 512:(n + 1) * 512],
                                    start=(k == 0), stop=(k == KT - 1))
                            g = gxs.tile([128, 512], F32R)
                            nc.vector.tensor_add(
                                g[:], ps[:], bias_bc[:, n * 512:(n + 1) * 512])
                            nc.sync.dma_start(
                                gx_d[m * 128:(m + 1) * 128,
                                     n * 512:(n + 1) * 512], g[:])

                # P1c: recurrence
                with tc.tile_pool(name=f"wh{l}", bufs=1) as whp, \
                     tc.tile_pool(name=f"st{l}", bufs=1) as st, \
                     tc.tile_pool(name=f"gq{l}", bufs=2) as gq, \
                     tc.tile_pool(name=f"rp{l}", bufs=3) as rp, \
                     tc.tile_pool(name=f"gps{l}", bufs=5, space="PSUM") as gps, \
                     tc.tile_pool(name=f"tps{l}", bufs=2, space="PSUM") as tps:
                    wh = []
                    for k in range(KT):
                        w = whp.tile([128, H4], F32R, tag=f"wh{k}", name=f"wht{l}_{k}")
                        nc.sync.dma_start(w[:], wh_d[l, k * 128:(k + 1) * 128, :])
                        wh.append(w)
                    c_sb = st.tile([32, H], F32, tag="c_sb")
                    nc.vector.memset(c_sb, 0.0)

                    for t in range(T):
                        gx_t = gq.tile([32, H4], F32R)
                        nc.sync.dma_start(gx_t[:],
                                          gx_d[t * 32:(t + 1) * 32, :])
                        for n in range(NCH):
                            ps = gps.tile([32, 512], F32)
                            for k in range(KT):
                                hT = (z32 if t == 0
                                      else Bt[k][:, (t - 1) * 32:t * 32])
                                nc.tensor.matmul(
                                    ps[:], hT,
                                    wh[k][:, n * 512:(n + 1) * 512],
                                    start=(k == 0), stop=False)
                            nc.tensor.matmul(
                                ps[:], ident[:],
                                gx_t[:, n * 512:(n + 1) * 512],
                                start=False, stop=True)
                            # gate math; chunk layout [i|f|o|g] x128
                            ifo = rp.tile([32, 384], F32)
                            nc.scalar.activation(ifo[:], ps[:, 0:384],
                                                 AF.Sigmoid)
                            gg = rp.tile([32, 128], F32)
                            nc.scalar.activation(gg[:], ps[:, 384:512],
                                                 AF.Tanh)
                            t1 = rp.tile([32, 128], F32)
                            nc.vector.tensor_mul(t1[:], ifo[:, 0:128], gg[:])
                            t2 = rp.tile([32, 128], F32)
                            nc.vector.tensor_mul(
                                t2[:], ifo[:, 128:256],
                                c_sb[:, n * 128:(n + 1) * 128])
                            nc.vector.tensor_add(
                                c_sb[:, n * 128:(n + 1) * 128], t1[:], t2[:])
                            th = rp.tile([32, 128], F32)
                            nc.scalar.activation(
                                th[:], c_sb[:, n * 128:(n + 1) * 128], AF.Tanh)
                            hch = rp.tile([32, 128], F32R)
                            nc.vector.tensor_mul(hch[:], ifo[:, 256:384],
                                                 th[:])
                            tp = tps.tile([128, 32], F32R)
                            nc.tensor.transpose(tp[:], hch[:], ident[:])
                            nc.vector.tensor_copy(
                                Bt[n][:, t * 32:(t + 1) * 32], tp[:])
            A, Bt = Bt, A

        # ---- P2: output projection ----
        with tc.tile_pool(name="p2", bufs=1) as p2, \
             tc.tile_pool(name="p2s", bufs=4) as p2s, \
             tc.tile_pool(name="p2ps", bufs=4, space="PSUM") as p2ps:
            wo = []
            for k in range(KT):
                w = p2.tile([128, OUT], F32R, tag=f"wo{k}", name=f"wot{k}")
                nc.sync.dma_start(w[:], wo_d[k * 128:(k + 1) * 128, :])
                wo.append(w)
            obt = p2.tile([OUT, 1], F32, tag="obt")
            nc.sync.dma_start(obt[:], ob_d)
            for n in range(R // 512):
                ps = p2ps.tile([OUT, 512], F32)
                for k in range(KT):
                    nc.tensor.matmul(ps[:], wo[k][:],
                                     A[k][:, n * 512:(n + 1) * 512],
                                     start=(k == 0), stop=(k == KT - 1))
                y = p2s.tile([OUT, 512], F32)
                nc.scalar.activation(y[:], ps[:], AF.Identity, bias=obt[:])
                nc.sync.dma_start(yT_d[:, n * 512:(n + 1) * 512], y[:])

        for p in reversed(ctxpools):
            p.release()

    nc.compile()
    return nc


def _get_prog():
    global _PROG
    if _PROG is None:
        _PROG = _build()
    return _PROG


def _stage_inputs(x, proj_w, proj_b, lstm_w, lstm_b, out_w, out_b):
    perm = _gate_perm()
    lb = np.asarray(lstm_b, np.float32).copy()
    lb[:, 2 * H:3 * H] += 1.0          # forget-gate +1.0 folded into bias
    shared = {
        "pw": np.ascontiguousarray(np.asarray(proj_w, np.float32)),
        "pb": np.ascontiguousarray(np.asarray(proj_b, np.float32).reshape(H, 1)),
        "wx": np.ascontiguousarray(np.asarray(lstm_w, np.float32)[:, :H, :][:, :, perm]),
        "wh": np.ascontiguousarray(np.asarray(lstm_w, np.float32)[:, H:, :][:, :, perm]),
        "bi": np.ascontiguousarray(lb[:, perm]),
        "wo": np.ascontiguousarray(np.asarray(out_w, np.float32)),
        "ob": np.ascontiguousarray(np.asarray(out_b, np.float32).reshape(OUT, 1)),
    }
    x = np.asarray(x, np.float32)
    in_maps = []
    for c in range(NCORES):
        xs = x[c * BL:(c + 1) * BL]                     # [32, 128, 16]
        xT = np.ascontiguousarray(xs.transpose(2, 1, 0).reshape(F_IN, R))
        in_maps.append({"xT": xT, **shared})
    return in_maps


def kernel(x, proj_w, proj_b, lstm_w, lstm_b, out_w, out_b, _trace=False):
    nc = _get_prog()
    in_maps = _stage_inputs(x, proj_w, proj_b, lstm_w, lstm_b, out_w, out_b)
    res = run_bass_kernel_spmd(nc, in_maps, core_ids=list(range(NCORES)),
                               trace=_trace)
    y = np.empty((B, T, OUT), np.float32)
    for c in range(NCORES):
        yT = res.results[c]["yT"]                       # [16, 4096]
        y[c * BL:(c + 1) * BL] = yT.reshape(OUT, T, BL).transpose(2, 1, 0)
    kernel._last_results = res
    return y



# revision 7
# speedup vs baseline: 2.4527x; 2.4527x over previous
"""Trainium2 Bass kernel for a 3-layer LSTM recurrent encoder.

Contract: kernel(**inputs) takes FULL inputs (as produced by
setup_inputs()) and returns the FULL output [256, 128, 16] fp32.

Strategy: data-parallel over batch (256 tracks -> 8 cores x 32), with the
recurrence computed in "orientation B": gates live in PSUM as
[128 gate-dims, 16 blocks x 32 tracks], i.e. gates transposed. Per 128-col
gate block g, step t:
    PS_t[:, g*32:(g+1)*32] = gxT_t  (DVE inject from SBUF staging)
                           += sum_k Wh[k,g-block]^T @ hT_{t-1}[k]
Stationary = Wh tile [128,128] bf16, moving = hT [128,32] bf16 -> measured
~34-60ns per matmul at saturation (vs 216ns for the A-orientation shape).
h comes out of the gate math already transposed ([h-dim, track]) so there
are NO PE transposes, and elementwise gate math runs on full 128-partition
tiles. The x-contribution (Wx part + bias) is computed 16 steps ahead in
batched 512-token matmuls (ACT epilogue applies the bias and casts to bf16
into an SBUF staging buffer); nothing round-trips through DRAM.

Layers: relu between layers is applied in-place on the h history two steps
behind the recurrence (after the last reader), so there is no batch relu
phase. Weights are double-buffered bf16; numerics: bf16 weights + bf16 h
gives ~6e-3 rel err (tolerance 2e-2).
"""

import sys

sys.path.insert(0, "/opt/trn_rl_repo")

import ml_dtypes
import numpy as np

import concourse.bacc as bacc
import concourse.mybir as mybir
import concourse.tile as tile
from concourse.bass_utils import run_bass_kernel_spmd

AF = mybir.ActivationFunctionType
F32 = mybir.dt.float32
BF16 = mybir.dt.bfloat16

B, T, F_IN = 256, 128, 16
H, L, OUT = 512, 3, 16
NCORES = 8
BL = B // NCORES          # 32 tracks per core
R = BL * T                # 4096 tokens per core
H4 = 4 * H                # 2048 gate columns
KT = H // 128             # 4 k-tiles of the hidden dim
NG = 16                   # gate blocks of 128 cols
SG = 16                   # steps per x-part staging group

_PROG = None


def _perm_b():
    """Permute gate cols from [i|g|f|o] (orig) to [i|f|o|g] block order."""
    return np.concatenate([
        np.arange(0, 512),          # i
        np.arange(1024, 1536),      # f
        np.arange(1536, 2048),      # o
        np.arange(512, 1024),       # g
    ])


def _build():
    nc = bacc.Bacc("TRN2", target_bir_lowering=False, debug=False,
                   num_devices=NCORES)

    xT_d = nc.dram_tensor("xT", [F_IN, R], BF16, kind="ExternalInput").ap()
    pw_d = nc.dram_tensor("pw", [F_IN, H], BF16, kind="ExternalInput").ap()
    pb_d = nc.dram_tensor("pb", [128, KT], F32, kind="ExternalInput").ap()
    wx_d = nc.dram_tensor("wx", [L, KT, 128, H4], BF16, kind="ExternalInput").ap()
    wh_d = nc.dram_tensor("wh", [L, KT, 128, H4], BF16, kind="ExternalInput").ap()
    bi_d = nc.dram_tensor("bi", [L, 128, NG], F32, kind="ExternalInput").ap()
    wo_d = nc.dram_tensor("wo", [KT, 128, OUT], BF16, kind="ExternalInput").ap()
    ob_d = nc.dram_tensor("ob", [OUT, 1], F32, kind="ExternalInput").ap()
    yT_d = nc.dram_tensor("yT", [OUT, R], F32, kind="ExternalOutput").ap()

    with tile.TileContext(nc) as tc:
        const = tc.alloc_tile_pool(name="const", bufs=1)
        z32b = const.tile([128, BL], BF16, tag="z32b")
        nc.vector.memset(z32b, 0.0)

        hA = tc.alloc_tile_pool(name="hA", bufs=1)
        hB = tc.alloc_tile_pool(name="hB", bufs=1)
        A = [hA.tile([128, R], BF16, tag=f"A{k}", name=f"A{k}") for k in range(KT)]
        Bt = [hB.tile([128, R], BF16, tag=f"B{k}", name=f"B{k}") for k in range(KT)]

        # weight slots (double buffered across layers)
        wts = tc.alloc_tile_pool(name="wts", bufs=1)
        wxt = [[wts.tile([128, H4], BF16, tag=f"wx{s}_{k}", name=f"wx{s}_{k}")
                for k in range(KT)] for s in range(2)]
        wht = [[wts.tile([128, H4], BF16, tag=f"wh{s}_{k}", name=f"wh{s}_{k}")
                for k in range(KT)] for s in range(2)]
        bit = [wts.tile([128, NG], F32, tag=f"bi{s}", name=f"bi{s}")
               for s in range(2)]
        # x-part staging buffers (one per 16-step group, ping-pong)
        gxt = [wts.tile([128, SG, 512], BF16, tag=f"gx{s}", name=f"gx{s}")
               for s in range(2)]

        def load_weights(l):
            s = l % 2
            for k in range(KT):
                nc.sync.dma_start(wxt[s][k][:], wx_d[l, k])
                nc.sync.dma_start(wht[s][k][:], wh_d[l, k])
            nc.sync.dma_start(bit[s][:], bi_d[l])

        load_weights(0)

        # ---- P0: projection -> A (bf16, relu) ----
        with tc.tile_pool(name="p0", bufs=1) as p0, \
             tc.tile_pool(name="p0ps", bufs=4, space="PSUM") as p0ps:
            xTt = p0.tile([F_IN, R], BF16, tag="xTt")
            nc.sync.dma_start(xTt[:], xT_d)
            pwt = p0.tile([F_IN, H], BF16, tag="pwt")
            nc.sync.dma_start(pwt[:], pw_d)
            pbt = p0.tile([128, KT], F32, tag="pbt")
            nc.sync.dma_start(pbt[:], pb_d)
            with nc.named_scope("P0"):
                for c in range(R // 512):
                    for k in range(KT):
                        ps = p0ps.tile([128, 512], F32)
                        nc.tensor.matmul(ps[:], pwt[:, k * 128:(k + 1) * 128],
                                         xTt[:, c * 512:(c + 1) * 512],
                                         start=True, stop=True)
                        nc.scalar.activation(A[k][:, c * 512:(c + 1) * 512],
                                             ps[:], AF.Relu,
                                             bias=pbt[:, k:k + 1])

        # ---- layers ----
        with tc.tile_pool(name="cs", bufs=2) as csp, \
             tc.tile_pool(name="sig", bufs=3) as sigp, \
             tc.tile_pool(name="tg", bufs=3) as tgp, \
             tc.tile_pool(name="tc_", bufs=3) as tcp, \
             tc.tile_pool(name="t12", bufs=4) as t12p, \
             tc.tile_pool(name="ps", bufs=4, space="PSUM") as psp, \
             tc.tile_pool(name="p1b", bufs=2, space="PSUM") as p1bp:

            for l in range(L):
                s = l % 2
                if l + 1 < L:
                    load_weights(l + 1)

                c_sb = csp.tile([128, 128], F32, name="c_sb")
                nc.vector.memset(c_sb, 0.0)

                def p1b_one(g, sg):
                    """x-part for staging group sg (tokens sg*512..), block g."""
                    tok0 = sg * 512
                    ps = p1bp.tile([128, SG, 32], F32, name="xp")
                    for k in range(KT):
                        nc.tensor.matmul(
                            ps[:], wxt[s][k][:, g * 128:(g + 1) * 128],
                            A[k][:, tok0:tok0 + 512],
                            start=(k == 0), stop=(k == KT - 1))
                    nc.scalar.activation(
                        gxt[sg % 2][:, :, g * 32:(g + 1) * 32], ps[:],
                        AF.Identity, bias=bit[s][:, g:g + 1])

                with nc.named_scope(f"L{l}head"):
                    for g in range(NG):
                        p1b_one(g, 0)

                ps_t = {}

                def inject(t):
                    ps = psp.tile([128, 512], F32, name="psg")
                    ps_t[t] = ps
                    nc.vector.tensor_copy(
                        ps[:], gxt[(t // SG) % 2][:, t % SG, :])

                inject(0)
                inject(1)

                with nc.named_scope(f"L{l}rec"):
                    for t in range(T):
                        ps = ps_t.pop(t)
                        # recurrence matmuls: k outer so h[k] consumed asap
                        for k in range(KT):
                            hsrc = (z32b[:] if t == 0
                                    else Bt[k][:, (t - 1) * BL:t * BL])
                            for g in range(NG):
                                nc.tensor.matmul(
                                    ps[:, g * 32:(g + 1) * 32],
                                    wht[s][k][:, g * 128:(g + 1) * 128],
                                    hsrc, start=False, stop=(k == KT - 1))
                        # prefetch next step's PSUM init (ahead of gate math
                        # in the DVE queue; runs while PE streams this step).
                        # At a staging-group boundary the init reads columns
                        # whose epilogue is emitted later this step, so the
                        # inject must be emitted after it (program order is
                        # what the dependency tracker sees).
                        if t + 1 < T and (t + 1) % SG != 0:
                            inject(t + 1)
                        # gate math; sig cols = [i(128) | f(128) | o(128)]
                        sig = sigp.tile([128, 384], F32)
                        nc.scalar.activation(sig[:], ps[:, 0:384], AF.Sigmoid)
                        tg = tgp.tile([128, 128], F32)
                        nc.scalar.activation(tg[:], ps[:, 384:512], AF.Tanh)
                        t1 = t12p.tile([128, 128], F32)
                        nc.vector.tensor_mul(t1[:], sig[:, 0:128], tg[:])
                        t2 = t12p.tile([128, 128], F32)
                        nc.vector.tensor_mul(t2[:], sig[:, 128:256], c_sb[:])
                        nc.vector.tensor_add(c_sb[:], t1[:], t2[:])
                        tc_ = tcp.tile([128, 128], F32)
                        nc.scalar.activation(tc_[:], c_sb[:], AF.Tanh)
                        for k in range(KT):
                            nc.vector.tensor_mul(
                                Bt[k][:, t * BL:(t + 1) * BL],
                                sig[:, 256 + k * 32:256 + (k + 1) * 32],
                                tc_[:, k * 32:(k + 1) * 32])
                        # deferred in-place relu (input to next layer); the
                        # raw h at t-2 was last read by step t-1's matmuls
                        if l + 1 < L and t >= 2:
                            tr = t - 2
                            for k in range(KT):
                                nc.scalar.activation(
                                    Bt[k][:, tr * BL:(tr + 1) * BL],
                                    Bt[k][:, tr * BL:(tr + 1) * BL], AF.Relu)
                        # x-part for steps 16 ahead (4 matmuls + 1 epilogue
                        # per step fills PE while it waits on the h chain)
                        if t < T - SG:
                            p1b_one(t % SG, t // SG + 1)
                        if t + 1 < T and (t + 1) % SG == 0:
                            inject(t + 1)

                    if l + 1 < L:
                        for tr in (T - 2, T - 1):
                            for k in range(KT):
                                nc.scalar.activation(
                                    Bt[k][:, tr * BL:(tr + 1) * BL],
                                    Bt[k][:, tr * BL:(tr + 1) * BL], AF.Relu)

                A, Bt = Bt, A

        # ---- P2: output projection ----
        with tc.tile_pool(name="p2", bufs=1) as p2, \
             tc.tile_pool(name="p2s", bufs=4) as p2s, \
             tc.tile_pool(name="p2ps", bufs=4, space="PSUM") as p2ps:
            wo = []
            for k in range(KT):
                w = p2.tile([128, OUT], BF16, tag=f"wo{k}", name=f"wot{k}")
                nc.sync.dma_start(w[:], wo_d[k])
                wo.append(w)
            obt = p2.tile([OUT, 1], F32, tag="obt")
            nc.sync.dma_start(obt[:], ob_d)
            with nc.named_scope("P2"):
                for c in range(R // 512):
                    ps = p2ps.tile([OUT, 512], F32)
                    for k in range(KT):
                        nc.tensor.matmul(ps[:], wo[k][:],
                                         A[k][:, c * 512:(c + 1) * 512],
                                         start=(k == 0), stop=(k == KT - 1))
                    y = p2s.tile([OUT, 512], F32)
                    nc.scalar.activation(y[:], ps[:], AF.Identity, bias=obt[:])
                    nc.sync.dma_start(yT_d[:, c * 512:(c + 1) * 512], y[:])

        for p in (wts, hB, hA, const):
            p.release()

    nc.compile()
    return nc


def _get_prog():
    global _PROG
    if _PROG is None:
        _PROG = _build()
    return _PROG


def _stage_inputs(x, proj_w, proj_b, lstm_w, lstm_b, out_w, out_b):
    perm = _perm_b()
    bf = ml_dtypes.bfloat16
    lw = np.asarray(lstm_w, np.float32)
    lb = np.asarray(lstm_b, np.float32).copy()
    lb[:, 2 * H:3 * H] += 1.0  # forget-gate +1.0 folded into bias
    shared = {
        "pw": np.ascontiguousarray(np.asarray(proj_w, np.float32)).astype(bf),
        "pb": np.ascontiguousarray(
            np.asarray(proj_b, np.float32).reshape(KT, 128).T),
        "wx": np.ascontiguousarray(
            lw[:, :H, :][:, :, perm].reshape(L, KT, 128, H4)).astype(bf),
        "wh": np.ascontiguousarray(
            lw[:, H:, :][:, :, perm].reshape(L, KT, 128, H4)).astype(bf),
        "bi": np.ascontiguousarray(
            lb[:, perm].reshape(L, NG, 128).transpose(0, 2, 1)),
        "wo": np.ascontiguousarray(
            np.asarray(out_w, np.float32).reshape(KT, 128, OUT)).astype(bf),
        "ob": np.ascontiguousarray(
            np.asarray(out_b, np.float32).reshape(OUT, 1)),
    }
    x = np.asarray(x, np.float32)
    in_maps = []
    for c in range(NCORES):
        xs = x[c * BL:(c + 1) * BL]                     # [32, 128, 16]
        xT = np.ascontiguousarray(
            xs.transpose(2, 1, 0).reshape(F_IN, R)).astype(bf)
        in_maps.append({"xT": xT, **shared})
    return in_maps


def kernel(x, proj_w, proj_b, lstm_w, lstm_b, out_w, out_b, _trace=False):
    nc = _get_prog()
    in_maps = _stage_inputs(x, proj_w, proj_b, lstm_w, lstm_b, out_w, out_b)
    res = run_bass_kernel_spmd(nc, in_maps, core_ids=list(range(NCORES)),
                               trace=_trace)
    y = np.empty((B, T, OUT), np.float32)
    for c in range(NCORES):
        yT = res.results[c]["yT"]                       # [16, 4096]
        y[c * BL:(c + 1) * BL] = yT.reshape(OUT, T, BL).transpose(2, 1, 0)
    kernel._last_results = res
    return y
